# revision 21
# baseline (speedup 1.0000x reference)
"""DeepSeekMoE forward on 8 Trainium2 NeuronCores (Bass/Tile).

Strategy (expert-parallel, host dispatch/combine):
  - Router (sigmoid scores + top-4 + gating) computed on host with jax-CPU,
    bitwise-matching the reference's op sequence.
  - 24 uniform "FFN jobs": 16 routed experts (tokens gathered per expert,
    padded per-slot) + 2 shared experts x 4 token-shards of 2048.
  - Each core runs 3 jobs: 1 shared-expert shard + 2 routed experts,
    paired largest-with-smallest so slot capacities are
    [2048, C0=pad(max count), C1=pad(9th-largest count)] instead of
    2x global max.
  - Per job: H^T = relu(W1^T X^T + b1); Y^T = W2^T H^T + b2, computed with
    feature-major bf16 matmuls (full-rate on trn2 PE, fp32 PSUM accum;
    bf16 also enables fast-weight-load so LDWEIGHTS hides under MMs).
  - Host scatters routed outputs back with gating weights and adds the
    residual + shared outputs.
"""

import numpy as np

D_MODEL, D_FF, NS, NR, KR = 2048, 1408, 2, 16, 4
P = 128
DT = D_MODEL // P  # 16
FT = D_FF // P     # 11
NCORES = 8
JOBS = 3           # per core: [shared shard, routed expert big, routed small]
SH_TOK = 2048      # shared-expert shard size (per core)

_prog_cache = {}
LAST_RESULT = None  # BassKernelResults of the most recent device run


def _ensure_ntff_hook():
    """This image's `antenv` lacks the `axon_hooks` get/set registry that
    `run_bass_kernel_spmd(trace=True)` imports under axon; install an
    equivalent shim backed by the libaxon ctypes profiler so tracing works
    (and BASS_TRACE=1 doesn't crash the run)."""
    try:
        from antenv.axon_hooks import get_axon_ntff_profile_hook  # noqa: F401
        return
    except ImportError:
        pass
    import sys
    import types
    try:
        import antenv
        mod = types.ModuleType("antenv.axon_hooks")
        _hook = [None]
        mod.set_axon_ntff_profile_hook = lambda h: _hook.__setitem__(0, h)
        mod.get_axon_ntff_profile_hook = lambda: _hook[0]
        sys.modules["antenv.axon_hooks"] = mod
        antenv.axon_hooks = mod
        from trn_agent_boot.trn_boot import _ntff_profile_via_ctypes
        mod.set_axon_ntff_profile_hook(
            _ntff_profile_via_ctypes("/opt/axon/libaxon_pjrt.so")
        )
    except Exception:
        pass


def _plan_chunks(block):
    """Split a block (multiple of 128) into moving-dim chunks in
    {128, 256, 384, 512} (PSUM bank is 512 fp32; bigger chunks amortize
    per-MM issue overhead)."""
    n8 = block // P
    assert block % P == 0 and n8 >= 1
    out = []
    while n8 > 0:
        if n8 in (1, 2, 3, 4):
            out.append(n8 * P)
            n8 = 0
        elif n8 == 5:
            out += [2 * P, 3 * P]
            n8 = 0
        else:
            out.append(4 * P)
            n8 -= 4
    return out


def _plan_blocks(C, first_small=False):
    """Split capacity C into token blocks of at most 1152 (SBUF budget),
    each a multiple of 128. first_small peels a 256-token block off the
    front so the pipeline primes with minimal DMA."""
    blocks = []
    rem = C
    if first_small and C > 768:
        # big enough that mm1 (one ring's worth of w1) covers the weight
        # DMA of the block, small enough to prime the pipeline fast
        blocks.append(384)
        rem -= 384
    while rem > 0:
        if rem <= 1152:
            blocks.append(rem)
            rem = 0
        elif rem - 1024 >= 256:
            blocks.append(1024)
            rem -= 1024
        else:
            b = (rem // 2 // P) * P
            blocks += [b, rem - b]
            rem = 0
    assert sum(blocks) == C and all(b >= P and b % P == 0 for b in blocks)
    return blocks


def _build_program(caps):
    import concourse.mybir as mybir
    import concourse.tile as tile
    from concourse import bacc

    F32 = mybir.dt.float32
    BF16 = mybir.dt.bfloat16
    Relu = mybir.ActivationFunctionType.Relu
    Identity = mybir.ActivationFunctionType.Identity

    job_tokens = list(caps)
    cmax = max(job_tokens)
    job_blocks = [
        _plan_blocks(t, first_small=(j == 0)) for j, t in enumerate(job_tokens)
    ]

    # X is packed per block, [P, DT*blk] contiguous per partition, so every
    # X DMA moves 32KB-contiguous partition lines (1-2KB lines measured only
    # ~100-200 GB/s per ring; packed runs at full ring rate).
    total_cols = DT * sum(sum(b) for b in job_blocks)
    nc = bacc.Bacc(None, target_bir_lowering=False)
    xt = nc.dram_tensor("xt", [P, total_cols], BF16, kind="ExternalInput")
    w1 = nc.dram_tensor("w1", [JOBS, FT, P, DT, P], BF16, kind="ExternalInput")
    w2 = nc.dram_tensor("w2", [JOBS, DT, P, FT, P], BF16, kind="ExternalInput")
    b1 = nc.dram_tensor("b1", [P, JOBS * FT], F32, kind="ExternalInput")
    b2 = nc.dram_tensor("b2", [P, JOBS * DT], F32, kind="ExternalInput")
    yt = nc.dram_tensor("yt", [JOBS, DT, P, cmax], BF16, kind="ExternalOutput")

    with tile.TileContext(nc) as tc:
        with (
            tc.tile_pool(name="const", bufs=1) as const,
            tc.tile_pool(name="h", bufs=1) as hpool,
            tc.tile_pool(name="w1p", bufs=10) as w1pool,
            tc.tile_pool(name="w2p", bufs=8) as w2pool,
            tc.tile_pool(name="y", bufs=4) as ypool,
            tc.tile_pool(name="ps", bufs=8, space="PSUM") as pspool,
        ):
            # biases ride the (otherwise idle) GPSIMD SW-DGE: tiny transfers
            # whose per-packet latency would delay block 0's X on a HW ring
            b1t = const.tile([P, JOBS * FT], F32)
            nc.gpsimd.dma_start(b1t[:], b1[:, :])
            b2t = const.tile([P, JOBS * DT], F32)
            nc.gpsimd.dma_start(b2t[:], b2[:, :])

            # HAM warm-up: the first real matmul can't start before its
            # 2MB X tile lands (~11us); idle-starting the PE then runs
            # everything at 1.2 GHz until ~3.4us of sustained busy flips
            # the clock gate to 8/8. Issue a burst of dummy matmuls with
            # NO dependencies (uninitialized SBUF into a scratch PSUM that
            # is never read) sized to end right as the X tile lands: the
            # gate is warm and the PE never idles. 22 MMs = ~3.4us cold +
            # ~3us warm after the ~7.7us engine preamble, ending just as
            # block 0's packed X lands (~14us).
            warm = const.tile([P, 512], BF16)
            nc.vector.memset(warm[:], 0.0)
            warm_ps = pspool.tile([P, 512], F32, tag="ps")
            for _ in range(22):
                nc.tensor.matmul(
                    warm_ps[:], warm[:, :P], warm[:], start=True, stop=True
                )

            def emit_block(j, off, blk, xts, chunks):
                h_t = hpool.tile([P, FT, blk], BF16, tag="h")
                for ft in range(FT):
                    if ft == 0:
                        w1_t = w1_firsts.pop(0)
                    else:
                        w1_t = w1pool.tile([P, DT, P], BF16, tag="w1")
                        nc.sync.dma_start(w1_t[:], w1[j, ft])
                    coff = 0
                    for ch in chunks:
                        ps = pspool.tile([P, 512], F32, tag="ps")
                        for ko in range(DT):
                            lhsT = (
                                w1_t[ko][:]
                                if isinstance(w1_t, list)
                                else w1_t[:, ko]
                            )
                            nc.tensor.matmul(
                                ps[:, :ch],
                                lhsT,
                                xts[ko][:, coff : coff + ch],
                                start=(ko == 0),
                                stop=(ko == DT - 1),
                            )
                        nc.scalar.activation(
                            h_t[:, ft, coff : coff + ch],
                            ps[:, :ch],
                            Relu,
                            bias=b1t[:, j * FT + ft : j * FT + ft + 1],
                        )
                        coff += ch

                for dtile in range(DT):
                    w2_t = w2pool.tile([P, FT, P], BF16, tag="w2")
                    # ACT HW-DGE ring: splits weight bandwidth with the SP
                    # ring (w1) — the early blocks otherwise outrun a
                    # single ring and re-throttle HAM. Deep w2 pool keeps
                    # the WAR sem always-clear so the scalar FIFO never
                    # stalls behind this descriptor.
                    nc.scalar.dma_start(w2_t[:], w2[j, dtile])
                    y_t = ypool.tile([P, 1152], BF16, tag="y")
                    coff = 0
                    for ch in chunks:
                        ps = pspool.tile([P, 512], F32, tag="ps")
                        for ko in range(FT):
                            nc.tensor.matmul(
                                ps[:, :ch],
                                w2_t[:, ko],
                                h_t[:, ko, coff : coff + ch],
                                start=(ko == 0),
                                stop=(ko == FT - 1),
                            )
                        nc.scalar.activation(
                            y_t[:, coff : coff + ch],
                            ps[:, :ch],
                            Identity,
                            bias=b2t[:, j * DT + dtile : j * DT + dtile + 1],
                        )
                        coff += ch
                    # Y rides the ACT HW-DGE ring: keeps the SP ring free
                    # for weight transfers.
                    nc.scalar.dma_start(
                        yt[j, dtile, :, off : off + blk], y_t[:, :blk]
                    )

            w1_firsts = []

            def load_w1_first(j):
                t = w1pool.tile([P, DT, P], BF16, tag="w1")
                nc.sync.dma_start(t[:], w1[j, 0])
                w1_firsts.append(t)

            # Steady state: X rides the SP ring, double-buffered (bufs=2)
            # and issued before the block's w1 loads, so it sits at most
            # behind the previous block's consumption-paced w1 tiles —
            # ~a full block of slack. (The GPSIMD SW-DGE alternative
            # moves X at only ~110 GB/s and showed 35us late-landing X.)
            # Exception: block 0's X goes on the ACT ring (idle at start)
            # so it lands in parallel with the w1 tiles on SP — one big
            # descriptor, since per-descriptor issue is ~0.6us and a
            # fine-grained priming split serializes on issue rate.
            with tc.tile_pool(name="x", bufs=2) as xpool:
                first = True
                xcol = 0
                for j in range(JOBS):
                    off = 0
                    for blk in job_blocks[j]:
                        chunks = _plan_chunks(blk)
                        xt_t = xpool.tile([P, DT, blk], BF16, tag="x")
                        ring = nc.scalar if first else nc.sync
                        first = False
                        ring.dma_start(
                            xt_t[:], xt[:, xcol : xcol + DT * blk]
                        )
                        xcol += DT * blk
                        load_w1_first(j)
                        emit_block(
                            j, off, blk,
                            [xt_t[:, ko] for ko in range(DT)], chunks,
                        )
                        off += blk

    nc.compile()
    return nc


def _routing(flat, centroids, bias):
    """Replicate the reference router bitwise: jax-CPU sigmoid scores,
    stable top-4 (argsort matches lax.top_k tie-breaking), normalized gates."""
    import jax
    import jax.numpy as jnp

    cpu = jax.devices("cpu")[0]
    with jax.default_device(cpu):
        scores = np.asarray(
            jax.nn.sigmoid(jnp.asarray(flat) @ jnp.asarray(centroids).T)
            + jnp.asarray(bias)
        )
    idx = np.argsort(-scores, axis=-1, kind="stable")[:, :KR]
    vals = np.take_along_axis(scores, idx, axis=-1)
    gating = vals / np.maximum(vals.sum(-1, keepdims=True, dtype=np.float32), 1e-8)
    return idx.astype(np.int32), gating.astype(np.float32)


def _feat_major(x_td):
    """[T, D] (rows=tokens) -> [P, D//P, T] feature-major device layout."""
    d = x_td.shape[1]
    return np.ascontiguousarray(x_td.T.reshape(d // P, P, -1).transpose(1, 0, 2))


def _w_tiles(w, kdim, mdim):
    """[K, M] -> [M//P, P(k_inner), K//P, P(m_inner)] lhsT tile layout."""
    kt, mt = kdim // P, mdim // P
    return np.ascontiguousarray(
        w.reshape(kt, P, mt, P).transpose(2, 1, 0, 3)
    )


def kernel(u, shared_w1, shared_b1, shared_w2, shared_b2,
           routed_w1, routed_b1, routed_w2, routed_b2, centroids, bias):
    import ml_dtypes
    from concourse.bass_utils import run_bass_kernel_spmd

    BF16 = np.dtype(ml_dtypes.bfloat16)

    _ensure_ntff_hook()
    u = np.asarray(u, dtype=np.float32)
    b, s, d = u.shape
    flat = u.reshape(-1, d)
    T = flat.shape[0]

    idx, gating = _routing(flat, np.asarray(centroids, np.float32),
                           np.asarray(bias, np.float32))

    # per-expert token lists (ascending token id) and their gate values
    tok_lists, gate_lists = [], []
    for e in range(NR):
        hit = idx == e                        # [T, KR]
        rows = np.nonzero(hit.any(axis=1))[0]
        g = gating[hit].reshape(-1)           # row-major -> ascending token id
        tok_lists.append(rows)
        gate_lists.append(g.astype(np.float32))

    # Pair largest with smallest so slot capacities are
    # C0 = pad(count of largest), C1 = pad(9th-largest count).
    counts = np.array([len(r) for r in tok_lists])
    order = np.argsort(-counts, kind="stable")
    pad = lambda c: max(256, -(-c // P) * P)
    C0 = pad(counts[order[0]])
    C1 = pad(counts[order[NCORES]])
    caps = (SH_TOK, C0, C1)
    cmax = max(caps)
    job_blocks = [
        _plan_blocks(t, first_small=(j == 0)) for j, t in enumerate(caps)
    ]
    total_cols = DT * sum(sum(bl) for bl in job_blocks)

    if caps not in _prog_cache:
        _prog_cache[caps] = _build_program(caps)
    nc = _prog_cache[caps]

    flat_bf = flat.astype(BF16)
    sw1 = np.asarray(shared_w1, np.float32).astype(BF16)
    sb1 = np.asarray(shared_b1, np.float32)
    sw2 = np.asarray(shared_w2, np.float32).astype(BF16)
    sb2 = np.asarray(shared_b2, np.float32)
    rw1 = np.asarray(routed_w1, np.float32).astype(BF16)
    rb1 = np.asarray(routed_b1, np.float32)
    rw2 = np.asarray(routed_w2, np.float32).astype(BF16)
    rb2 = np.asarray(routed_b2, np.float32)

    rw1_t = [_w_tiles(rw1[e], D_MODEL, D_FF) for e in range(NR)]
    rw2_t = [_w_tiles(rw2[e], D_FF, D_MODEL) for e in range(NR)]
    sw1_t = [_w_tiles(sw1[n], D_MODEL, D_FF) for n in range(NS)]
    sw2_t = [_w_tiles(sw2[n], D_FF, D_MODEL) for n in range(NS)]

    in_maps = []
    core_experts = []
    for core in range(NCORES):
        sh_e = core % NS
        sh_off = (core // NS) * SH_TOK
        e0 = int(order[core])                 # big expert -> slot 1 (C0)
        e1 = int(order[2 * NCORES - 1 - core])  # small expert -> slot 2 (C1)
        core_experts.append((e0, e1))

        fms = []
        fm0 = np.zeros((P, DT, SH_TOK), BF16)
        fm0[:] = _feat_major(flat_bf[sh_off : sh_off + SH_TOK])
        fms.append(fm0)
        for jslot, e in ((1, e0), (2, e1)):
            fmj = np.zeros((P, DT, caps[jslot]), BF16)
            rows = tok_lists[e]
            if len(rows):
                fmj[:, :, : len(rows)] = _feat_major(flat_bf[rows])
            fms.append(fmj)

        # pack X per block, [P, DT*blk] contiguous, in program block order
        xt = np.empty((P, total_cols), BF16)
        col = 0
        for j in range(JOBS):
            off = 0
            for blk in job_blocks[j]:
                xt[:, col : col + DT * blk] = fms[j][:, :, off : off + blk
                                                    ].reshape(P, DT * blk)
                col += DT * blk
                off += blk

        w1 = np.stack([sw1_t[sh_e], rw1_t[e0], rw1_t[e1]])
        w2 = np.stack([sw2_t[sh_e], rw2_t[e0], rw2_t[e1]])
        b1m = np.stack([sb1[sh_e], rb1[e0], rb1[e1]])   # [3, 1408]
        b2m = np.stack([sb2[sh_e], rb2[e0], rb2[e1]])   # [3, 2048]
        b1m = np.ascontiguousarray(b1m.reshape(JOBS * FT, P).T)  # [P, 33]
        b2m = np.ascontiguousarray(b2m.reshape(JOBS * DT, P).T)  # [P, 48]

        in_maps.append({"xt": xt, "w1": w1, "w2": w2, "b1": b1m, "b2": b2m})

    res = run_bass_kernel_spmd(nc, in_maps, core_ids=list(range(NCORES)))
    global LAST_RESULT
    LAST_RESULT = res

    out = flat.copy()
    for core in range(NCORES):
        ytc = res.results[core]["yt"]          # [JOBS, DT, P, cmax] bf16
        sh_off = (core // NS) * SH_TOK
        out[sh_off : sh_off + SH_TOK] += (
            ytc[0].reshape(D_MODEL, cmax)[:, :SH_TOK].T.astype(np.float32)
        )
        e0, e1 = core_experts[core]
        for jslot, e in ((1, e0), (2, e1)):
            rows = tok_lists[e]
            if len(rows):
                ye = ytc[jslot].reshape(D_MODEL, cmax)[:, : len(rows)].T
                out[rows] += gate_lists[e][:, None] * ye.astype(np.float32)

    return out.reshape(b, s, d)


# revision 26
# speedup vs baseline: 1.0130x; 1.0130x over previous
"""DeepSeekMoE forward on 8 Trainium2 NeuronCores (Bass/Tile).

Strategy (expert-parallel, host dispatch/combine):
  - Router (sigmoid scores + top-4 + gating) computed on host with jax-CPU,
    bitwise-matching the reference's op sequence.
  - 24 uniform "FFN jobs": 16 routed experts (tokens gathered per expert,
    padded per-slot) + 2 shared experts x 4 token-shards of 2048.
  - Each core runs 3 jobs: 1 shared-expert shard + 2 routed experts,
    paired largest-with-smallest so slot capacities are
    [2048, C0=pad(max count), C1=pad(9th-largest count)] instead of
    2x global max.
  - Per job: H^T = relu(W1^T X^T + b1); Y^T = W2^T H^T + b2, computed with
    feature-major bf16 matmuls (full-rate on trn2 PE, fp32 PSUM accum;
    bf16 also enables fast-weight-load so LDWEIGHTS hides under MMs).
  - Host scatters routed outputs back with gating weights and adds the
    residual + shared outputs.
"""

import numpy as np

D_MODEL, D_FF, NS, NR, KR = 2048, 1408, 2, 16, 4
P = 128
DT = D_MODEL // P  # 16
FT = D_FF // P     # 11
NCORES = 8
JOBS = 3           # per core: [shared shard, routed expert big, routed small]
SH_TOK = 2048      # shared-expert shard size (per core)

_prog_cache = {}
LAST_RESULT = None  # BassKernelResults of the most recent device run


def _ensure_ntff_hook():
    """This image's `antenv` lacks the `axon_hooks` get/set registry that
    `run_bass_kernel_spmd(trace=True)` imports under axon; install an
    equivalent shim backed by the libaxon ctypes profiler so tracing works
    (and BASS_TRACE=1 doesn't crash the run)."""
    try:
        from antenv.axon_hooks import get_axon_ntff_profile_hook  # noqa: F401
        return
    except ImportError:
        pass
    import sys
    import types
    try:
        import antenv
        mod = types.ModuleType("antenv.axon_hooks")
        _hook = [None]
        mod.set_axon_ntff_profile_hook = lambda h: _hook.__setitem__(0, h)
        mod.get_axon_ntff_profile_hook = lambda: _hook[0]
        sys.modules["antenv.axon_hooks"] = mod
        antenv.axon_hooks = mod
        from trn_agent_boot.trn_boot import _ntff_profile_via_ctypes
        mod.set_axon_ntff_profile_hook(
            _ntff_profile_via_ctypes("/opt/axon/libaxon_pjrt.so")
        )
    except Exception:
        pass


def _plan_chunks(block):
    """Split a block (multiple of 128) into moving-dim chunks in
    {128, 256, 384, 512} (PSUM bank is 512 fp32; bigger chunks amortize
    per-MM issue overhead)."""
    n8 = block // P
    assert block % P == 0 and n8 >= 1
    out = []
    while n8 > 0:
        if n8 in (1, 2, 3, 4):
            out.append(n8 * P)
            n8 = 0
        elif n8 == 5:
            out += [2 * P, 3 * P]
            n8 = 0
        else:
            out.append(4 * P)
            n8 -= 4
    return out


def _plan_blocks(C, first_small=False):
    """Split capacity C into token blocks of at most 1152 (SBUF budget),
    each a multiple of 128. first_small peels a 256-token block off the
    front so the pipeline primes with minimal DMA."""
    blocks = []
    rem = C
    if first_small and C > 768:
        # big enough that mm1 (one ring's worth of w1) covers the weight
        # DMA of the block, small enough to prime the pipeline fast
        blocks.append(512)
        rem -= 512
    while rem > 0:
        if rem <= 1152:
            blocks.append(rem)
            rem = 0
        elif rem - 1024 >= 256:
            blocks.append(1024)
            rem -= 1024
        else:
            b = (rem // 2 // P) * P
            blocks += [b, rem - b]
            rem = 0
    assert sum(blocks) == C and all(b >= P and b % P == 0 for b in blocks)
    return blocks


def _build_program(caps):
    import concourse.mybir as mybir
    import concourse.tile as tile
    from concourse import bacc

    F32 = mybir.dt.float32
    BF16 = mybir.dt.bfloat16
    Relu = mybir.ActivationFunctionType.Relu
    Identity = mybir.ActivationFunctionType.Identity

    job_tokens = list(caps)
    cmax = max(job_tokens)
    job_blocks = [
        _plan_blocks(t, first_small=(j == 0)) for j, t in enumerate(job_tokens)
    ]

    # X is packed per block, [P, DT*blk] contiguous per partition, so every
    # X DMA moves 32KB-contiguous partition lines (1-2KB lines measured only
    # ~100-200 GB/s per ring; packed runs at full ring rate).
    total_cols = DT * sum(sum(b) for b in job_blocks)
    nc = bacc.Bacc(None, target_bir_lowering=False)
    xt = nc.dram_tensor("xt", [P, total_cols], BF16, kind="ExternalInput")
    w1 = nc.dram_tensor("w1", [JOBS, FT, P, DT, P], BF16, kind="ExternalInput")
    w2 = nc.dram_tensor("w2", [JOBS, DT, P, FT, P], BF16, kind="ExternalInput")
    b1 = nc.dram_tensor("b1", [P, JOBS * FT], F32, kind="ExternalInput")
    b2 = nc.dram_tensor("b2", [P, JOBS * DT], F32, kind="ExternalInput")
    yt = nc.dram_tensor("yt", [JOBS, DT, P, cmax], BF16, kind="ExternalOutput")

    with tile.TileContext(nc) as tc:
        with (
            tc.tile_pool(name="const", bufs=1) as const,
            tc.tile_pool(name="h", bufs=1) as hpool,
            # weights stay SBUF-resident for a whole job (one load per
            # job, not per block): per-ft/dtile tags with bufs=1 give one
            # buffer per tile, so job j+1's tiles WAR-wait on job j's
            # last use and the transfers trickle in during job j's last
            # block.
            tc.tile_pool(name="w1p", bufs=1) as w1pool,
            tc.tile_pool(name="w2p", bufs=1) as w2pool,
            tc.tile_pool(name="y", bufs=4) as ypool,
            tc.tile_pool(name="ps", bufs=8, space="PSUM") as pspool,
        ):
            # biases ride the (otherwise idle) GPSIMD SW-DGE: tiny transfers
            # whose per-packet latency would delay block 0's X on a HW ring
            b1t = const.tile([P, JOBS * FT], F32)
            nc.gpsimd.dma_start(b1t[:], b1[:, :])
            b2t = const.tile([P, JOBS * DT], F32)
            nc.gpsimd.dma_start(b2t[:], b2[:, :])

            # HAM warm-up: the first real matmul can't start before its
            # 2MB X tile lands (~11us); idle-starting the PE then runs
            # everything at 1.2 GHz until ~3.4us of sustained busy flips
            # the clock gate to 8/8. Issue a burst of dummy matmuls with
            # NO dependencies (uninitialized SBUF into a scratch PSUM that
            # is never read) sized to end right as the X tile lands: the
            # gate is warm and the PE never idles. 22 MMs = ~3.4us cold +
            # ~3us warm after the ~7.7us engine preamble, ending just as
            # block 0's packed X lands (~14us).
            warm = const.tile([P, 512], BF16)
            nc.vector.memset(warm[:], 0.0)
            warm_ps = pspool.tile([P, 512], F32, tag="ps")
            for _ in range(22):
                nc.tensor.matmul(
                    warm_ps[:], warm[:, :P], warm[:], start=True, stop=True
                )

            def emit_block(j, off, blk, xts, chunks, w1ts, w2ts):
                h_t = hpool.tile([P, FT, blk], BF16, tag="h")
                for ft in range(FT):
                    coff = 0
                    for ch in chunks:
                        ps = pspool.tile([P, 512], F32, tag="ps")
                        for ko in range(DT):
                            nc.tensor.matmul(
                                ps[:, :ch],
                                w1ts[ft][:, ko],
                                xts[ko][:, coff : coff + ch],
                                start=(ko == 0),
                                stop=(ko == DT - 1),
                            )
                        nc.scalar.activation(
                            h_t[:, ft, coff : coff + ch],
                            ps[:, :ch],
                            Relu,
                            bias=b1t[:, j * FT + ft : j * FT + ft + 1],
                        )
                        coff += ch

                for dtile in range(DT):
                    y_t = ypool.tile([P, 1152], BF16, tag="y")
                    coff = 0
                    for ch in chunks:
                        ps = pspool.tile([P, 512], F32, tag="ps")
                        for ko in range(FT):
                            nc.tensor.matmul(
                                ps[:, :ch],
                                w2ts[dtile][:, ko],
                                h_t[:, ko, coff : coff + ch],
                                start=(ko == 0),
                                stop=(ko == FT - 1),
                            )
                        nc.scalar.activation(
                            y_t[:, coff : coff + ch],
                            ps[:, :ch],
                            Identity,
                            bias=b2t[:, j * DT + dtile : j * DT + dtile + 1],
                        )
                        coff += ch
                    # Y rides the ACT HW-DGE ring: keeps the SP ring free
                    # for weight transfers.
                    nc.scalar.dma_start(
                        yt[j, dtile, :, off : off + blk], y_t[:, :blk]
                    )

            # Steady state: X rides the SP ring, double-buffered (bufs=2);
            # w1 on SP, w2 on ACT (splits weight bandwidth), each loaded
            # ONCE per job and SBUF-resident across its blocks. Block 0's
            # X goes on the ACT ring (idle at start) so it lands in
            # parallel with job 0's w1 tiles on SP — one big descriptor,
            # since per-descriptor issue is ~0.6us and a fine-grained
            # priming split serializes on issue rate.
            with tc.tile_pool(name="x", bufs=2) as xpool:
                first = True
                xcol = 0
                for j in range(JOBS):
                    # job 0, block 0: X must be FIRST on the ACT ring so
                    # priming isn't stuck behind the 5.6MB w2 preload
                    xt_first = None
                    if first:
                        blk = job_blocks[0][0]
                        xt_first = xpool.tile([P, DT, blk], BF16, tag="x")
                        nc.scalar.dma_start(
                            xt_first[:], xt[:, 0 : DT * blk]
                        )
                        xcol = DT * blk
                    w1ts = []
                    w2ts = []
                    for ft in range(FT):
                        t = w1pool.tile([P, DT, P], BF16, tag=f"w1_{ft}")
                        nc.sync.dma_start(t[:], w1[j, ft])
                        w1ts.append(t)
                    for dtile in range(DT):
                        t = w2pool.tile([P, FT, P], BF16, tag=f"w2_{dtile}")
                        nc.scalar.dma_start(t[:], w2[j, dtile])
                        w2ts.append(t)
                    off = 0
                    for blk in job_blocks[j]:
                        chunks = _plan_chunks(blk)
                        if xt_first is not None:
                            xt_t, xt_first = xt_first, None
                        else:
                            xt_t = xpool.tile([P, DT, blk], BF16, tag="x")
                            nc.sync.dma_start(
                                xt_t[:], xt[:, xcol : xcol + DT * blk]
                            )
                            xcol += DT * blk
                        first = False
                        emit_block(
                            j, off, blk,
                            [xt_t[:, ko] for ko in range(DT)], chunks,
                            w1ts, w2ts,
                        )
                        off += blk

    nc.compile()
    return nc


def _routing(flat, centroids, bias):
    """Replicate the reference router bitwise: jax-CPU sigmoid scores,
    stable top-4 (argsort matches lax.top_k tie-breaking), normalized gates."""
    import jax
    import jax.numpy as jnp

    cpu = jax.devices("cpu")[0]
    with jax.default_device(cpu):
        scores = np.asarray(
            jax.nn.sigmoid(jnp.asarray(flat) @ jnp.asarray(centroids).T)
            + jnp.asarray(bias)
        )
    idx = np.argsort(-scores, axis=-1, kind="stable")[:, :KR]
    vals = np.take_along_axis(scores, idx, axis=-1)
    gating = vals / np.maximum(vals.sum(-1, keepdims=True, dtype=np.float32), 1e-8)
    return idx.astype(np.int32), gating.astype(np.float32)


def _feat_major(x_td):
    """[T, D] (rows=tokens) -> [P, D//P, T] feature-major device layout."""
    d = x_td.shape[1]
    return np.ascontiguousarray(x_td.T.reshape(d // P, P, -1).transpose(1, 0, 2))


def _w_tiles(w, kdim, mdim):
    """[K, M] -> [M//P, P(k_inner), K//P, P(m_inner)] lhsT tile layout."""
    kt, mt = kdim // P, mdim // P
    return np.ascontiguousarray(
        w.reshape(kt, P, mt, P).transpose(2, 1, 0, 3)
    )


def kernel(u, shared_w1, shared_b1, shared_w2, shared_b2,
           routed_w1, routed_b1, routed_w2, routed_b2, centroids, bias):
    import ml_dtypes
    from concourse.bass_utils import run_bass_kernel_spmd

    BF16 = np.dtype(ml_dtypes.bfloat16)

    _ensure_ntff_hook()
    u = np.asarray(u, dtype=np.float32)
    b, s, d = u.shape
    flat = u.reshape(-1, d)
    T = flat.shape[0]

    idx, gating = _routing(flat, np.asarray(centroids, np.float32),
                           np.asarray(bias, np.float32))

    # per-expert token lists (ascending token id) and their gate values
    tok_lists, gate_lists = [], []
    for e in range(NR):
        hit = idx == e                        # [T, KR]
        rows = np.nonzero(hit.any(axis=1))[0]
        g = gating[hit].reshape(-1)           # row-major -> ascending token id
        tok_lists.append(rows)
        gate_lists.append(g.astype(np.float32))

    # Pair largest with smallest so slot capacities are
    # C0 = pad(count of largest), C1 = pad(9th-largest count).
    counts = np.array([len(r) for r in tok_lists])
    order = np.argsort(-counts, kind="stable")
    pad = lambda c: max(256, -(-c // P) * P)
    C0 = pad(counts[order[0]])
    C1 = pad(counts[order[NCORES]])
    caps = (SH_TOK, C0, C1)
    cmax = max(caps)
    job_blocks = [
        _plan_blocks(t, first_small=(j == 0)) for j, t in enumerate(caps)
    ]
    total_cols = DT * sum(sum(bl) for bl in job_blocks)

    if caps not in _prog_cache:
        _prog_cache[caps] = _build_program(caps)
    nc = _prog_cache[caps]

    flat_bf = flat.astype(BF16)
    sw1 = np.asarray(shared_w1, np.float32).astype(BF16)
    sb1 = np.asarray(shared_b1, np.float32)
    sw2 = np.asarray(shared_w2, np.float32).astype(BF16)
    sb2 = np.asarray(shared_b2, np.float32)
    rw1 = np.asarray(routed_w1, np.float32).astype(BF16)
    rb1 = np.asarray(routed_b1, np.float32)
    rw2 = np.asarray(routed_w2, np.float32).astype(BF16)
    rb2 = np.asarray(routed_b2, np.float32)

    rw1_t = [_w_tiles(rw1[e], D_MODEL, D_FF) for e in range(NR)]
    rw2_t = [_w_tiles(rw2[e], D_FF, D_MODEL) for e in range(NR)]
    sw1_t = [_w_tiles(sw1[n], D_MODEL, D_FF) for n in range(NS)]
    sw2_t = [_w_tiles(sw2[n], D_FF, D_MODEL) for n in range(NS)]

    in_maps = []
    core_experts = []
    for core in range(NCORES):
        sh_e = core % NS
        sh_off = (core // NS) * SH_TOK
        e0 = int(order[core])                 # big expert -> slot 1 (C0)
        e1 = int(order[2 * NCORES - 1 - core])  # small expert -> slot 2 (C1)
        core_experts.append((e0, e1))

        fms = []
        fm0 = np.zeros((P, DT, SH_TOK), BF16)
        fm0[:] = _feat_major(flat_bf[sh_off : sh_off + SH_TOK])
        fms.append(fm0)
        for jslot, e in ((1, e0), (2, e1)):
            fmj = np.zeros((P, DT, caps[jslot]), BF16)
            rows = tok_lists[e]
            if len(rows):
                fmj[:, :, : len(rows)] = _feat_major(flat_bf[rows])
            fms.append(fmj)

        # pack X per block, [P, DT*blk] contiguous, in program block order
        xt = np.empty((P, total_cols), BF16)
        col = 0
        for j in range(JOBS):
            off = 0
            for blk in job_blocks[j]:
                xt[:, col : col + DT * blk] = fms[j][:, :, off : off + blk
                                                    ].reshape(P, DT * blk)
                col += DT * blk
                off += blk

        w1 = np.stack([sw1_t[sh_e], rw1_t[e0], rw1_t[e1]])
        w2 = np.stack([sw2_t[sh_e], rw2_t[e0], rw2_t[e1]])
        b1m = np.stack([sb1[sh_e], rb1[e0], rb1[e1]])   # [3, 1408]
        b2m = np.stack([sb2[sh_e], rb2[e0], rb2[e1]])   # [3, 2048]
        b1m = np.ascontiguousarray(b1m.reshape(JOBS * FT, P).T)  # [P, 33]
        b2m = np.ascontiguousarray(b2m.reshape(JOBS * DT, P).T)  # [P, 48]

        in_maps.append({"xt": xt, "w1": w1, "w2": w2, "b1": b1m, "b2": b2m})

    res = run_bass_kernel_spmd(nc, in_maps, core_ids=list(range(NCORES)))
    global LAST_RESULT
    LAST_RESULT = res

    out = flat.copy()
    for core in range(NCORES):
        ytc = res.results[core]["yt"]          # [JOBS, DT, P, cmax] bf16
        sh_off = (core // NS) * SH_TOK
        out[sh_off : sh_off + SH_TOK] += (
            ytc[0].reshape(D_MODEL, cmax)[:, :SH_TOK].T.astype(np.float32)
        )
        e0, e1 = core_experts[core]
        for jslot, e in ((1, e0), (2, e1)):
            rows = tok_lists[e]
            if len(rows):
                ye = ytc[jslot].reshape(D_MODEL, cmax)[:, : len(rows)].T
                out[rows] += gate_lists[e][:, None] * ye.astype(np.float32)

    return out.reshape(b, s, d)


# revision 30
# speedup vs baseline: 1.0183x; 1.0052x over previous
"""DeepSeekMoE forward on 8 Trainium2 NeuronCores (Bass/Tile).

Strategy (expert-parallel, host dispatch/combine):
  - Router (sigmoid scores + top-4 + gating) computed on host with jax-CPU,
    bitwise-matching the reference's op sequence.
  - 24 uniform "FFN jobs": 16 routed experts (tokens gathered per expert,
    padded per-slot) + 2 shared experts x 4 token-shards of 2048.
  - Each core runs 3 jobs: 1 shared-expert shard + 2 routed experts,
    paired largest-with-smallest so slot capacities are
    [2048, C0=pad(max count), C1=pad(9th-largest count)] instead of
    2x global max.
  - Per job: H^T = relu(W1^T X^T + b1); Y^T = W2^T H^T + b2, computed with
    feature-major bf16 matmuls (full-rate on trn2 PE, fp32 PSUM accum;
    bf16 also enables fast-weight-load so LDWEIGHTS hides under MMs).
  - Host scatters routed outputs back with gating weights and adds the
    residual + shared outputs.
"""

import numpy as np

D_MODEL, D_FF, NS, NR, KR = 2048, 1408, 2, 16, 4
P = 128
DT = D_MODEL // P  # 16
FT = D_FF // P     # 11
NCORES = 8
JOBS = 3           # per core: [shared shard, routed expert big, routed small]
SH_TOK = 2048      # shared-expert shard size (per core)

_prog_cache = {}
LAST_RESULT = None  # BassKernelResults of the most recent device run


def _ensure_ntff_hook():
    """This image's `antenv` lacks the `axon_hooks` get/set registry that
    `run_bass_kernel_spmd(trace=True)` imports under axon; install an
    equivalent shim backed by the libaxon ctypes profiler so tracing works
    (and BASS_TRACE=1 doesn't crash the run)."""
    try:
        from antenv.axon_hooks import get_axon_ntff_profile_hook  # noqa: F401
        return
    except ImportError:
        pass
    import sys
    import types
    try:
        import antenv
        mod = types.ModuleType("antenv.axon_hooks")
        _hook = [None]
        mod.set_axon_ntff_profile_hook = lambda h: _hook.__setitem__(0, h)
        mod.get_axon_ntff_profile_hook = lambda: _hook[0]
        sys.modules["antenv.axon_hooks"] = mod
        antenv.axon_hooks = mod
        from trn_agent_boot.trn_boot import _ntff_profile_via_ctypes
        mod.set_axon_ntff_profile_hook(
            _ntff_profile_via_ctypes("/opt/axon/libaxon_pjrt.so")
        )
    except Exception:
        pass


def _plan_chunks(block):
    """Split a block (multiple of 128) into moving-dim chunks in
    {128, 256, 384, 512} (PSUM bank is 512 fp32; bigger chunks amortize
    per-MM issue overhead)."""
    n8 = block // P
    assert block % P == 0 and n8 >= 1
    out = []
    while n8 > 0:
        if n8 in (1, 2, 3, 4):
            out.append(n8 * P)
            n8 = 0
        elif n8 == 5:
            out += [2 * P, 3 * P]
            n8 = 0
        else:
            out.append(4 * P)
            n8 -= 4
    return out


def _plan_blocks(C, first_small=False):
    """Split capacity C into token blocks of at most 1152 (SBUF budget),
    each a multiple of 128. first_small peels a 256-token block off the
    front so the pipeline primes with minimal DMA."""
    blocks = []
    rem = C
    if first_small and C > 768:
        # big enough that mm1 (one ring's worth of w1) covers the weight
        # DMA of the block, small enough to prime the pipeline fast
        blocks.append(512)
        rem -= 512
    while rem > 0:
        if rem <= 1152:
            blocks.append(rem)
            rem = 0
        elif rem - 1024 >= 256:
            blocks.append(1024)
            rem -= 1024
        else:
            b = (rem // 2 // P) * P
            blocks += [b, rem - b]
            rem = 0
    assert sum(blocks) == C and all(b >= P and b % P == 0 for b in blocks)
    return blocks


def _build_program(caps):
    import concourse.mybir as mybir
    import concourse.tile as tile
    from concourse import bacc

    F32 = mybir.dt.float32
    BF16 = mybir.dt.bfloat16
    Relu = mybir.ActivationFunctionType.Relu
    Identity = mybir.ActivationFunctionType.Identity

    job_tokens = list(caps)
    cmax = max(job_tokens)
    job_blocks = [
        _plan_blocks(t, first_small=(j == 0)) for j, t in enumerate(job_tokens)
    ]

    # X is packed per block, [P, DT*blk] contiguous per partition, so every
    # X DMA moves 32KB-contiguous partition lines (1-2KB lines measured only
    # ~100-200 GB/s per ring; packed runs at full ring rate).
    total_cols = DT * sum(sum(b) for b in job_blocks)
    nc = bacc.Bacc(None, target_bir_lowering=False)
    xt = nc.dram_tensor("xt", [P, total_cols], BF16, kind="ExternalInput")
    w1 = nc.dram_tensor("w1", [JOBS, FT, P, DT, P], BF16, kind="ExternalInput")
    w2 = nc.dram_tensor("w2", [JOBS, DT, P, FT, P], BF16, kind="ExternalInput")
    b1 = nc.dram_tensor("b1", [P, JOBS * FT], F32, kind="ExternalInput")
    b2 = nc.dram_tensor("b2", [P, JOBS * DT], F32, kind="ExternalInput")
    yt = nc.dram_tensor("yt", [JOBS, DT, P, cmax], BF16, kind="ExternalOutput")

    with tile.TileContext(nc) as tc:
        with (
            tc.tile_pool(name="const", bufs=1) as const,
            tc.tile_pool(name="h", bufs=1) as hpool,
            # weights stay SBUF-resident for a whole job (one load per
            # job, not per block): per-ft/dtile tags with bufs=1 give one
            # buffer per tile, so job j+1's tiles WAR-wait on job j's
            # last use and the transfers trickle in during job j's last
            # block.
            tc.tile_pool(name="w1p", bufs=1) as w1pool,
            tc.tile_pool(name="w2p", bufs=1) as w2pool,
            tc.tile_pool(name="y", bufs=4) as ypool,
            tc.tile_pool(name="ps", bufs=8, space="PSUM") as pspool,
        ):
            # biases ride the (otherwise idle) GPSIMD SW-DGE: tiny transfers
            # whose per-packet latency would delay block 0's X on a HW ring
            b1t = const.tile([P, JOBS * FT], F32)
            nc.gpsimd.dma_start(b1t[:], b1[:, :])
            b2t = const.tile([P, JOBS * DT], F32)
            nc.gpsimd.dma_start(b2t[:], b2[:, :])

            # HAM warm-up: the first real matmul can't start before its
            # 2MB X tile lands (~11us); idle-starting the PE then runs
            # everything at 1.2 GHz until ~3.4us of sustained busy flips
            # the clock gate to 8/8. Issue a burst of dummy matmuls with
            # NO dependencies (uninitialized SBUF into a scratch PSUM that
            # is never read) sized to end right as the X tile lands: the
            # gate is warm and the PE never idles. 22 MMs = ~3.4us cold +
            # ~3us warm after the ~7.7us engine preamble, ending just as
            # block 0's packed X lands (~14us).
            warm = const.tile([P, 512], BF16)
            nc.vector.memset(warm[:], 0.0)
            warm_ps = pspool.tile([P, 512], F32, tag="ps")
            for _ in range(22):
                nc.tensor.matmul(
                    warm_ps[:], warm[:, :P], warm[:], start=True, stop=True
                )

            def emit_block(j, off, blk, xts, chunks, w1ts, w2ts):
                h_t = hpool.tile([P, FT, blk], BF16, tag="h")
                for ft in range(FT):
                    coff = 0
                    for ch in chunks:
                        ps = pspool.tile([P, 512], F32, tag="ps")
                        for ko in range(DT):
                            nc.tensor.matmul(
                                ps[:, :ch],
                                w1ts[ft][:, ko],
                                xts[ko][:, coff : coff + ch],
                                start=(ko == 0),
                                stop=(ko == DT - 1),
                            )
                        nc.scalar.activation(
                            h_t[:, ft, coff : coff + ch],
                            ps[:, :ch],
                            Relu,
                            bias=b1t[:, j * FT + ft : j * FT + ft + 1],
                        )
                        coff += ch

                for dtile in range(DT):
                    y_t = ypool.tile([P, 1152], BF16, tag="y")
                    coff = 0
                    for ch in chunks:
                        ps = pspool.tile([P, 512], F32, tag="ps")
                        for ko in range(FT):
                            nc.tensor.matmul(
                                ps[:, :ch],
                                w2ts[dtile][:, ko],
                                h_t[:, ko, coff : coff + ch],
                                start=(ko == 0),
                                stop=(ko == FT - 1),
                            )
                        nc.scalar.activation(
                            y_t[:, coff : coff + ch],
                            ps[:, :ch],
                            Identity,
                            bias=b2t[:, j * DT + dtile : j * DT + dtile + 1],
                        )
                        coff += ch
                    # Y rides the ACT HW-DGE ring: keeps the SP ring free
                    # for weight transfers.
                    nc.scalar.dma_start(
                        yt[j, dtile, :, off : off + blk], y_t[:, :blk]
                    )

            # Steady state: X rides the SP ring, double-buffered (bufs=2);
            # w1 on SP, w2 on ACT (splits weight bandwidth), each loaded
            # ONCE per job and SBUF-resident across its blocks. Block 0's
            # X goes on the ACT ring (idle at start) so it lands in
            # parallel with job 0's w1 tiles on SP — one big descriptor,
            # since per-descriptor issue is ~0.6us and a fine-grained
            # priming split serializes on issue rate.
            with tc.tile_pool(name="x", bufs=2) as xpool:
                first = True
                xcol = 0
                for j in range(JOBS):
                    # job 0, block 0: X is issued FIRST, split half/half
                    # across BOTH rings (ahead of the w1/w2 preloads), so
                    # the priming X lands at aggregate ring rate (~12.5us)
                    # instead of serializing on one ring.
                    xts_first = None
                    if first:
                        blk = job_blocks[0][0]
                        hd = DT // 2
                        x0 = xpool.tile([P, DT, blk], BF16, tag="x")
                        nc.scalar.dma_start(
                            x0[:, 0:hd], xt[:, 0 : hd * blk]
                        )
                        nc.sync.dma_start(
                            x0[:, hd:DT], xt[:, hd * blk : DT * blk]
                        )
                        xts_first = [x0[:, ko] for ko in range(DT)]
                        xcol = DT * blk
                    w1ts = []
                    w2ts = []
                    for ft in range(FT):
                        t = w1pool.tile([P, DT, P], BF16, tag=f"w1_{ft}")
                        nc.sync.dma_start(t[:], w1[j, ft])
                        w1ts.append(t)
                    for dtile in range(DT):
                        t = w2pool.tile([P, FT, P], BF16, tag=f"w2_{dtile}")
                        nc.scalar.dma_start(t[:], w2[j, dtile])
                        w2ts.append(t)
                    off = 0
                    for blk in job_blocks[j]:
                        chunks = _plan_chunks(blk)
                        if xts_first is not None:
                            xts, xts_first = xts_first, None
                        else:
                            xt_t = xpool.tile([P, DT, blk], BF16, tag="x")
                            nc.sync.dma_start(
                                xt_t[:], xt[:, xcol : xcol + DT * blk]
                            )
                            xcol += DT * blk
                            xts = [xt_t[:, ko] for ko in range(DT)]
                        first = False
                        emit_block(
                            j, off, blk, xts, chunks, w1ts, w2ts,
                        )
                        off += blk

    nc.compile()
    return nc


def _routing(flat, centroids, bias):
    """Replicate the reference router bitwise: jax-CPU sigmoid scores,
    stable top-4 (argsort matches lax.top_k tie-breaking), normalized gates."""
    import jax
    import jax.numpy as jnp

    cpu = jax.devices("cpu")[0]
    with jax.default_device(cpu):
        scores = np.asarray(
            jax.nn.sigmoid(jnp.asarray(flat) @ jnp.asarray(centroids).T)
            + jnp.asarray(bias)
        )
    idx = np.argsort(-scores, axis=-1, kind="stable")[:, :KR]
    vals = np.take_along_axis(scores, idx, axis=-1)
    gating = vals / np.maximum(vals.sum(-1, keepdims=True, dtype=np.float32), 1e-8)
    return idx.astype(np.int32), gating.astype(np.float32)


def _feat_major(x_td):
    """[T, D] (rows=tokens) -> [P, D//P, T] feature-major device layout."""
    d = x_td.shape[1]
    return np.ascontiguousarray(x_td.T.reshape(d // P, P, -1).transpose(1, 0, 2))


def _w_tiles(w, kdim, mdim):
    """[K, M] -> [M//P, P(k_inner), K//P, P(m_inner)] lhsT tile layout."""
    kt, mt = kdim // P, mdim // P
    return np.ascontiguousarray(
        w.reshape(kt, P, mt, P).transpose(2, 1, 0, 3)
    )


def kernel(u, shared_w1, shared_b1, shared_w2, shared_b2,
           routed_w1, routed_b1, routed_w2, routed_b2, centroids, bias):
    import ml_dtypes
    from concourse.bass_utils import run_bass_kernel_spmd

    BF16 = np.dtype(ml_dtypes.bfloat16)

    _ensure_ntff_hook()
    u = np.asarray(u, dtype=np.float32)
    b, s, d = u.shape
    flat = u.reshape(-1, d)
    T = flat.shape[0]

    idx, gating = _routing(flat, np.asarray(centroids, np.float32),
                           np.asarray(bias, np.float32))

    # per-expert token lists (ascending token id) and their gate values
    tok_lists, gate_lists = [], []
    for e in range(NR):
        hit = idx == e                        # [T, KR]
        rows = np.nonzero(hit.any(axis=1))[0]
        g = gating[hit].reshape(-1)           # row-major -> ascending token id
        tok_lists.append(rows)
        gate_lists.append(g.astype(np.float32))

    # Pair largest with smallest so slot capacities are
    # C0 = pad(count of largest), C1 = pad(9th-largest count).
    counts = np.array([len(r) for r in tok_lists])
    order = np.argsort(-counts, kind="stable")
    pad = lambda c: max(256, -(-c // P) * P)
    C0 = pad(counts[order[0]])
    C1 = pad(counts[order[NCORES]])
    caps = (SH_TOK, C0, C1)
    cmax = max(caps)
    job_blocks = [
        _plan_blocks(t, first_small=(j == 0)) for j, t in enumerate(caps)
    ]
    total_cols = DT * sum(sum(bl) for bl in job_blocks)

    if caps not in _prog_cache:
        _prog_cache[caps] = _build_program(caps)
    nc = _prog_cache[caps]

    flat_bf = flat.astype(BF16)
    sw1 = np.asarray(shared_w1, np.float32).astype(BF16)
    sb1 = np.asarray(shared_b1, np.float32)
    sw2 = np.asarray(shared_w2, np.float32).astype(BF16)
    sb2 = np.asarray(shared_b2, np.float32)
    rw1 = np.asarray(routed_w1, np.float32).astype(BF16)
    rb1 = np.asarray(routed_b1, np.float32)
    rw2 = np.asarray(routed_w2, np.float32).astype(BF16)
    rb2 = np.asarray(routed_b2, np.float32)

    rw1_t = [_w_tiles(rw1[e], D_MODEL, D_FF) for e in range(NR)]
    rw2_t = [_w_tiles(rw2[e], D_FF, D_MODEL) for e in range(NR)]
    sw1_t = [_w_tiles(sw1[n], D_MODEL, D_FF) for n in range(NS)]
    sw2_t = [_w_tiles(sw2[n], D_FF, D_MODEL) for n in range(NS)]

    in_maps = []
    core_experts = []
    for core in range(NCORES):
        sh_e = core % NS
        sh_off = (core // NS) * SH_TOK
        e0 = int(order[core])                 # big expert -> slot 1 (C0)
        e1 = int(order[2 * NCORES - 1 - core])  # small expert -> slot 2 (C1)
        core_experts.append((e0, e1))

        fms = []
        fm0 = np.zeros((P, DT, SH_TOK), BF16)
        fm0[:] = _feat_major(flat_bf[sh_off : sh_off + SH_TOK])
        fms.append(fm0)
        for jslot, e in ((1, e0), (2, e1)):
            fmj = np.zeros((P, DT, caps[jslot]), BF16)
            rows = tok_lists[e]
            if len(rows):
                fmj[:, :, : len(rows)] = _feat_major(flat_bf[rows])
            fms.append(fmj)

        # pack X per block, [P, DT*blk] contiguous, in program block order
        xt = np.empty((P, total_cols), BF16)
        col = 0
        for j in range(JOBS):
            off = 0
            for blk in job_blocks[j]:
                xt[:, col : col + DT * blk] = fms[j][:, :, off : off + blk
                                                    ].reshape(P, DT * blk)
                col += DT * blk
                off += blk

        w1 = np.stack([sw1_t[sh_e], rw1_t[e0], rw1_t[e1]])
        w2 = np.stack([sw2_t[sh_e], rw2_t[e0], rw2_t[e1]])
        b1m = np.stack([sb1[sh_e], rb1[e0], rb1[e1]])   # [3, 1408]
        b2m = np.stack([sb2[sh_e], rb2[e0], rb2[e1]])   # [3, 2048]
        b1m = np.ascontiguousarray(b1m.reshape(JOBS * FT, P).T)  # [P, 33]
        b2m = np.ascontiguousarray(b2m.reshape(JOBS * DT, P).T)  # [P, 48]

        in_maps.append({"xt": xt, "w1": w1, "w2": w2, "b1": b1m, "b2": b2m})

    res = run_bass_kernel_spmd(nc, in_maps, core_ids=list(range(NCORES)))
    global LAST_RESULT
    LAST_RESULT = res

    out = flat.copy()
    for core in range(NCORES):
        ytc = res.results[core]["yt"]          # [JOBS, DT, P, cmax] bf16
        sh_off = (core // NS) * SH_TOK
        out[sh_off : sh_off + SH_TOK] += (
            ytc[0].reshape(D_MODEL, cmax)[:, :SH_TOK].T.astype(np.float32)
        )
        e0, e1 = core_experts[core]
        for jslot, e in ((1, e0), (2, e1)):
            rows = tok_lists[e]
            if len(rows):
                ye = ytc[jslot].reshape(D_MODEL, cmax)[:, : len(rows)].T
                out[rows] += gate_lists[e][:, None] * ye.astype(np.float32)

    return out.reshape(b, s, d)


# revision 35
# speedup vs baseline: 1.2420x; 1.2197x over previous
"""DeepSeekMoE forward on 8 Trainium2 NeuronCores (Bass/Tile).

Strategy (expert-parallel, host dispatch/combine):
  - Router (sigmoid scores + top-4 + gating) computed on host with jax-CPU,
    bitwise-matching the reference's op sequence.
  - 24 uniform "FFN jobs": 16 routed experts (tokens gathered per expert,
    padded per-slot) + 2 shared experts x 4 token-shards of 2048.
  - Each core runs 3 jobs: 1 shared-expert shard + 2 routed experts,
    paired largest-with-smallest so slot capacities are
    [2048, C0=pad(max count), C1=pad(9th-largest count)] instead of
    2x global max.
  - Per job: H^T = relu(W1^T X^T + b1); Y^T = W2^T H^T + b2, computed with
    feature-major bf16 matmuls (full-rate on trn2 PE, fp32 PSUM accum;
    bf16 also enables fast-weight-load so LDWEIGHTS hides under MMs).
  - Host scatters routed outputs back with gating weights and adds the
    residual + shared outputs.
"""

import numpy as np

D_MODEL, D_FF, NS, NR, KR = 2048, 1408, 2, 16, 4
P = 128
DT = D_MODEL // P  # 16
FT = D_FF // P     # 11
NCORES = 8
JOBS = 3           # per core: [shared shard, routed expert big, routed small]
SH_TOK = 2048      # shared-expert shard size (per core)

_prog_cache = {}
LAST_RESULT = None  # BassKernelResults of the most recent device run


def _ensure_ntff_hook():
    """This image's `antenv` lacks the `axon_hooks` get/set registry that
    `run_bass_kernel_spmd(trace=True)` imports under axon; install an
    equivalent shim backed by the libaxon ctypes profiler so tracing works
    (and BASS_TRACE=1 doesn't crash the run)."""
    try:
        from antenv.axon_hooks import get_axon_ntff_profile_hook  # noqa: F401
        return
    except ImportError:
        pass
    import sys
    import types
    try:
        import antenv
        mod = types.ModuleType("antenv.axon_hooks")
        _hook = [None]
        mod.set_axon_ntff_profile_hook = lambda h: _hook.__setitem__(0, h)
        mod.get_axon_ntff_profile_hook = lambda: _hook[0]
        sys.modules["antenv.axon_hooks"] = mod
        antenv.axon_hooks = mod
        from trn_agent_boot.trn_boot import _ntff_profile_via_ctypes
        mod.set_axon_ntff_profile_hook(
            _ntff_profile_via_ctypes("/opt/axon/libaxon_pjrt.so")
        )
    except Exception:
        pass


def _plan_chunks(block):
    """Split a block (multiple of 128) into moving-dim chunks in
    {128, 256, 384, 512} (PSUM bank is 512 fp32; bigger chunks amortize
    per-MM issue overhead)."""
    n8 = block // P
    assert block % P == 0 and n8 >= 1
    out = []
    while n8 > 0:
        if n8 in (1, 2, 3, 4):
            out.append(n8 * P)
            n8 = 0
        elif n8 == 5:
            out += [2 * P, 3 * P]
            n8 = 0
        else:
            out.append(4 * P)
            n8 -= 4
    return out


def _plan_blocks(C, first_small=False):
    """Split capacity C into token blocks of at most 1152 (SBUF budget),
    each a multiple of 128. first_small peels a 256-token block off the
    front so the pipeline primes with minimal DMA."""
    blocks = []
    rem = C
    if first_small and C > 768:
        # big enough that mm1 (one ring's worth of w1) covers the weight
        # DMA of the block, small enough to prime the pipeline fast
        blocks.append(512)
        rem -= 512
    while rem > 0:
        if rem <= 1152:
            blocks.append(rem)
            rem = 0
        elif rem - 1024 >= 256:
            blocks.append(1024)
            rem -= 1024
        else:
            b = (rem // 2 // P) * P
            blocks += [b, rem - b]
            rem = 0
    assert sum(blocks) == C and all(b >= P and b % P == 0 for b in blocks)
    return blocks


WSCALE = 64.0  # fp8 weights are pre-scaled by this; undone in ACTIVATE


def _build_program(caps):
    import concourse.mybir as mybir
    import concourse.tile as tile
    from concourse import bacc

    F32 = mybir.dt.float32
    BF16 = mybir.dt.bfloat16
    FP8 = mybir.dt.float8e4
    DR = mybir.MatmulPerfMode.DoubleRow
    Relu = mybir.ActivationFunctionType.Relu
    Identity = mybir.ActivationFunctionType.Identity

    job_tokens = list(caps)
    cmax = max(job_tokens)
    job_blocks = [
        _plan_blocks(t, first_small=(j == 0)) for j, t in enumerate(job_tokens)
    ]

    # X is packed per block, [P, DT*blk] contiguous per partition, so every
    # X DMA moves 32KB-contiguous partition lines (1-2KB lines measured only
    # ~100-200 GB/s per ring; packed runs at full ring rate).
    # Shared job (0) runs bf16; routed jobs (1,2) run fp8e4 DoubleRow
    # (~1.4-1.7x PE throughput; verified 1.3e-2 rel err vs the 2e-2 gate).
    colsb = DT * sum(job_blocks[0])
    colsr = DT * (sum(job_blocks[1]) + sum(job_blocks[2]))
    nc = bacc.Bacc(None, target_bir_lowering=False)
    xtb = nc.dram_tensor("xtb", [P, colsb], BF16, kind="ExternalInput")
    xtr = nc.dram_tensor("xtr", [P, colsr], FP8, kind="ExternalInput")
    w1b = nc.dram_tensor("w1b", [FT, P, DT, P], BF16, kind="ExternalInput")
    w2b = nc.dram_tensor("w2b", [DT, P, FT, P], BF16, kind="ExternalInput")
    w1r = nc.dram_tensor("w1r", [2, FT, P, DT, P], FP8, kind="ExternalInput")
    w2r = nc.dram_tensor("w2r", [2, DT, P, FT, P], FP8, kind="ExternalInput")
    b1 = nc.dram_tensor("b1", [P, JOBS * FT], F32, kind="ExternalInput")
    b2 = nc.dram_tensor("b2", [P, JOBS * DT], F32, kind="ExternalInput")
    yt = nc.dram_tensor("yt", [JOBS, DT, P, cmax], BF16, kind="ExternalOutput")

    with tile.TileContext(nc) as tc:
        with (
            tc.tile_pool(name="const", bufs=1) as const,
            tc.tile_pool(name="h", bufs=1) as hpool,
            # weights stay SBUF-resident for a whole job (one load per
            # job, not per block): per-ft/dtile tags with bufs=1 give one
            # buffer per tile, so job j+1's tiles WAR-wait on job j's
            # last use and the transfers trickle in during job j's last
            # block.
            tc.tile_pool(name="w1p", bufs=1) as w1pool,
            tc.tile_pool(name="w2p", bufs=1) as w2pool,
            tc.tile_pool(name="y", bufs=4) as ypool,
            tc.tile_pool(name="ps", bufs=8, space="PSUM") as pspool,
        ):
            # biases ride the (otherwise idle) GPSIMD SW-DGE: tiny transfers
            # whose per-packet latency would delay block 0's X on a HW ring
            b1t = const.tile([P, JOBS * FT], F32)
            nc.gpsimd.dma_start(b1t[:], b1[:, :])
            b2t = const.tile([P, JOBS * DT], F32)
            nc.gpsimd.dma_start(b2t[:], b2[:, :])

            # HAM warm-up: the first real matmul can't start before its
            # 2MB X tile lands (~11us); idle-starting the PE then runs
            # everything at 1.2 GHz until ~3.4us of sustained busy flips
            # the clock gate to 8/8. Issue a burst of dummy matmuls with
            # NO dependencies (uninitialized SBUF into a scratch PSUM that
            # is never read) sized to end right as the X tile lands: the
            # gate is warm and the PE never idles. 22 MMs = ~3.4us cold +
            # ~3us warm after the ~7.7us engine preamble, ending just as
            # block 0's packed X lands (~14us).
            warm = const.tile([P, 512], BF16)
            nc.vector.memset(warm[:], 0.0)
            warm_ps = pspool.tile([P, 512], F32, tag="ps")
            for _ in range(22):
                nc.tensor.matmul(
                    warm_ps[:], warm[:, :P], warm[:], start=True, stop=True
                )

            def emit_block(j, off, blk, x_t, chunks, w1ts, w2ts, fp8):
                h_t = hpool.tile([P, FT, blk], FP8 if fp8 else BF16, tag="h")
                scale = (1.0 / WSCALE) if fp8 else 1.0
                for ft in range(FT):
                    coff = 0
                    for ch in chunks:
                        ps = pspool.tile([P, 512], F32, tag="ps")
                        if fp8:
                            # DoubleRow: K=256 per MM via [P, 2, *] pairs
                            for pr in range(DT // 2):
                                nc.tensor.matmul(
                                    ps[:, :ch],
                                    w1ts[ft][:, 2 * pr : 2 * pr + 2],
                                    x_t[:, 2 * pr : 2 * pr + 2,
                                        coff : coff + ch],
                                    start=(pr == 0),
                                    stop=(pr == DT // 2 - 1),
                                    perf_mode=DR,
                                )
                        else:
                            for ko in range(DT):
                                nc.tensor.matmul(
                                    ps[:, :ch],
                                    w1ts[ft][:, ko],
                                    x_t[ko][:, coff : coff + ch],
                                    start=(ko == 0),
                                    stop=(ko == DT - 1),
                                )
                        nc.scalar.activation(
                            h_t[:, ft, coff : coff + ch],
                            ps[:, :ch],
                            Relu,
                            bias=b1t[:, j * FT + ft : j * FT + ft + 1],
                            scale=scale,
                        )
                        coff += ch

                for dtile in range(DT):
                    y_t = ypool.tile([P, 1152], BF16, tag="y")
                    coff = 0
                    for ch in chunks:
                        ps = pspool.tile([P, 512], F32, tag="ps")
                        if fp8:
                            # FT=11: 5 DoubleRow pairs + 1 normal fp8 MM
                            for pr in range(FT // 2):
                                nc.tensor.matmul(
                                    ps[:, :ch],
                                    w2ts[dtile][:, 2 * pr : 2 * pr + 2],
                                    h_t[:, 2 * pr : 2 * pr + 2,
                                        coff : coff + ch],
                                    start=(pr == 0),
                                    stop=False,
                                    perf_mode=DR,
                                )
                            nc.tensor.matmul(
                                ps[:, :ch],
                                w2ts[dtile][:, FT - 1],
                                h_t[:, FT - 1, coff : coff + ch],
                                start=False,
                                stop=True,
                            )
                        else:
                            for ko in range(FT):
                                nc.tensor.matmul(
                                    ps[:, :ch],
                                    w2ts[dtile][:, ko],
                                    h_t[:, ko, coff : coff + ch],
                                    start=(ko == 0),
                                    stop=(ko == FT - 1),
                                )
                        nc.scalar.activation(
                            y_t[:, coff : coff + ch],
                            ps[:, :ch],
                            Identity,
                            bias=b2t[:, j * DT + dtile : j * DT + dtile + 1],
                            scale=scale,
                        )
                        coff += ch
                    # Y rides the ACT HW-DGE ring: keeps the SP ring free
                    # for weight transfers.
                    nc.scalar.dma_start(
                        yt[j, dtile, :, off : off + blk], y_t[:, :blk]
                    )

            # Steady state: X rides the SP ring, double-buffered (bufs=2);
            # w1 on SP, w2 on ACT (splits weight bandwidth), each loaded
            # ONCE per job and SBUF-resident across its blocks. Block 0's
            # X goes on the ACT ring (idle at start) so it lands in
            # parallel with job 0's w1 tiles on SP — one big descriptor,
            # since per-descriptor issue is ~0.6us and a fine-grained
            # priming split serializes on issue rate.
            with tc.tile_pool(name="x", bufs=2) as xpool:
                first = True
                xcol = 0
                for j in range(JOBS):
                    fp8 = j > 0
                    xsrc = xtr if fp8 else xtb
                    if fp8 and j == 1:
                        xcol = 0
                    # job 0, block 0: X is issued FIRST, split half/half
                    # across BOTH rings (ahead of the w1/w2 preloads), so
                    # the priming X lands at aggregate ring rate (~12.5us)
                    # instead of serializing on one ring.
                    xts_first = None
                    if first:
                        blk = job_blocks[0][0]
                        hd = DT // 2
                        x0 = xpool.tile([P, DT, blk], BF16, tag="x")
                        nc.scalar.dma_start(
                            x0[:, 0:hd], xtb[:, 0 : hd * blk]
                        )
                        nc.sync.dma_start(
                            x0[:, hd:DT], xtb[:, hd * blk : DT * blk]
                        )
                        xts_first = [x0[:, ko] for ko in range(DT)]
                        xcol = DT * blk
                    w1ts = []
                    w2ts = []
                    wdt = FP8 if fp8 else BF16
                    for ft in range(FT):
                        t = w1pool.tile([P, DT, P], wdt, tag=f"w1_{ft}")
                        src = w1r[j - 1, ft] if fp8 else w1b[ft]
                        nc.sync.dma_start(t[:], src)
                        w1ts.append(t)
                    for dtile in range(DT):
                        t = w2pool.tile([P, FT, P], wdt, tag=f"w2_{dtile}")
                        src = w2r[j - 1, dtile] if fp8 else w2b[dtile]
                        nc.scalar.dma_start(t[:], src)
                        w2ts.append(t)
                    off = 0
                    for blk in job_blocks[j]:
                        chunks = _plan_chunks(blk)
                        if xts_first is not None:
                            x_arg, xts_first = xts_first, None
                        else:
                            xt_t = xpool.tile(
                                [P, DT, blk], FP8 if fp8 else BF16, tag="x"
                            )
                            nc.sync.dma_start(
                                xt_t[:], xsrc[:, xcol : xcol + DT * blk]
                            )
                            xcol += DT * blk
                            x_arg = (
                                xt_t if fp8
                                else [xt_t[:, ko] for ko in range(DT)]
                            )
                        first = False
                        emit_block(
                            j, off, blk, x_arg, chunks, w1ts, w2ts, fp8,
                        )
                        off += blk

    nc.compile()
    return nc


def _routing(flat, centroids, bias):
    """Replicate the reference router bitwise: jax-CPU sigmoid scores,
    stable top-4 (argsort matches lax.top_k tie-breaking), normalized gates."""
    import jax
    import jax.numpy as jnp

    cpu = jax.devices("cpu")[0]
    with jax.default_device(cpu):
        scores = np.asarray(
            jax.nn.sigmoid(jnp.asarray(flat) @ jnp.asarray(centroids).T)
            + jnp.asarray(bias)
        )
    idx = np.argsort(-scores, axis=-1, kind="stable")[:, :KR]
    vals = np.take_along_axis(scores, idx, axis=-1)
    gating = vals / np.maximum(vals.sum(-1, keepdims=True, dtype=np.float32), 1e-8)
    return idx.astype(np.int32), gating.astype(np.float32)


def _feat_major(x_td):
    """[T, D] (rows=tokens) -> [P, D//P, T] feature-major device layout."""
    d = x_td.shape[1]
    return np.ascontiguousarray(x_td.T.reshape(d // P, P, -1).transpose(1, 0, 2))


def _w_tiles(w, kdim, mdim):
    """[K, M] -> [M//P, P(k_inner), K//P, P(m_inner)] lhsT tile layout."""
    kt, mt = kdim // P, mdim // P
    return np.ascontiguousarray(
        w.reshape(kt, P, mt, P).transpose(2, 1, 0, 3)
    )


def kernel(u, shared_w1, shared_b1, shared_w2, shared_b2,
           routed_w1, routed_b1, routed_w2, routed_b2, centroids, bias):
    import ml_dtypes
    from concourse.bass_utils import run_bass_kernel_spmd

    BF16 = np.dtype(ml_dtypes.bfloat16)

    _ensure_ntff_hook()
    u = np.asarray(u, dtype=np.float32)
    b, s, d = u.shape
    flat = u.reshape(-1, d)
    T = flat.shape[0]

    idx, gating = _routing(flat, np.asarray(centroids, np.float32),
                           np.asarray(bias, np.float32))

    # per-expert token lists (ascending token id) and their gate values
    tok_lists, gate_lists = [], []
    for e in range(NR):
        hit = idx == e                        # [T, KR]
        rows = np.nonzero(hit.any(axis=1))[0]
        g = gating[hit].reshape(-1)           # row-major -> ascending token id
        tok_lists.append(rows)
        gate_lists.append(g.astype(np.float32))

    # Pair largest with smallest so slot capacities are
    # C0 = pad(count of largest), C1 = pad(9th-largest count).
    counts = np.array([len(r) for r in tok_lists])
    order = np.argsort(-counts, kind="stable")
    pad = lambda c: max(256, -(-c // P) * P)
    C0 = pad(counts[order[0]])
    C1 = pad(counts[order[NCORES]])
    caps = (SH_TOK, C0, C1)
    cmax = max(caps)
    job_blocks = [
        _plan_blocks(t, first_small=(j == 0)) for j, t in enumerate(caps)
    ]
    total_cols = DT * sum(sum(bl) for bl in job_blocks)

    if caps not in _prog_cache:
        _prog_cache[caps] = _build_program(caps)
    nc = _prog_cache[caps]

    FP8 = np.dtype(ml_dtypes.float8_e4m3)
    flat_bf = flat.astype(BF16)
    flat_q8 = flat.astype(FP8)
    sw1 = np.asarray(shared_w1, np.float32).astype(BF16)
    sb1 = np.asarray(shared_b1, np.float32)
    sw2 = np.asarray(shared_w2, np.float32).astype(BF16)
    sb2 = np.asarray(shared_b2, np.float32)
    rw1 = (np.asarray(routed_w1, np.float32) * WSCALE).astype(FP8)
    rb1 = np.asarray(routed_b1, np.float32)
    rw2 = (np.asarray(routed_w2, np.float32) * WSCALE).astype(FP8)
    rb2 = np.asarray(routed_b2, np.float32)

    rw1_t = [_w_tiles(rw1[e], D_MODEL, D_FF) for e in range(NR)]
    rw2_t = [_w_tiles(rw2[e], D_FF, D_MODEL) for e in range(NR)]
    sw1_t = [_w_tiles(sw1[n], D_MODEL, D_FF) for n in range(NS)]
    sw2_t = [_w_tiles(sw2[n], D_FF, D_MODEL) for n in range(NS)]

    in_maps = []
    core_experts = []
    for core in range(NCORES):
        sh_e = core % NS
        sh_off = (core // NS) * SH_TOK
        e0 = int(order[core])                 # big expert -> slot 1 (C0)
        e1 = int(order[2 * NCORES - 1 - core])  # small expert -> slot 2 (C1)
        core_experts.append((e0, e1))

        fms = []
        fm0 = np.zeros((P, DT, SH_TOK), BF16)
        fm0[:] = _feat_major(flat_bf[sh_off : sh_off + SH_TOK])
        fms.append(fm0)
        for jslot, e in ((1, e0), (2, e1)):
            fmj = np.zeros((P, DT, caps[jslot]), FP8)
            rows = tok_lists[e]
            if len(rows):
                fmj[:, :, : len(rows)] = _feat_major(flat_q8[rows])
            fms.append(fmj)

        # pack X per block, [P, DT*blk] contiguous, in program block order
        xtb = np.empty((P, DT * caps[0]), BF16)
        xtr = np.empty((P, DT * (caps[1] + caps[2])), FP8)
        colb = colr = 0
        for j in range(JOBS):
            off = 0
            for blk in job_blocks[j]:
                sl = fms[j][:, :, off : off + blk].reshape(P, DT * blk)
                if j == 0:
                    xtb[:, colb : colb + DT * blk] = sl
                    colb += DT * blk
                else:
                    xtr[:, colr : colr + DT * blk] = sl
                    colr += DT * blk
                off += blk

        b1m = np.stack([sb1[sh_e], rb1[e0], rb1[e1]])   # [3, 1408]
        b2m = np.stack([sb2[sh_e], rb2[e0], rb2[e1]])   # [3, 2048]
        b1m = np.ascontiguousarray(b1m.reshape(JOBS * FT, P).T)  # [P, 33]
        b2m = np.ascontiguousarray(b2m.reshape(JOBS * DT, P).T)  # [P, 48]

        in_maps.append({
            "xtb": xtb, "xtr": xtr,
            "w1b": sw1_t[sh_e], "w2b": sw2_t[sh_e],
            "w1r": np.stack([rw1_t[e0], rw1_t[e1]]),
            "w2r": np.stack([rw2_t[e0], rw2_t[e1]]),
            "b1": b1m, "b2": b2m,
        })

    res = run_bass_kernel_spmd(nc, in_maps, core_ids=list(range(NCORES)))
    global LAST_RESULT
    LAST_RESULT = res

    out = flat.copy()
    for core in range(NCORES):
        ytc = res.results[core]["yt"]          # [JOBS, DT, P, cmax] bf16
        sh_off = (core // NS) * SH_TOK
        out[sh_off : sh_off + SH_TOK] += (
            ytc[0].reshape(D_MODEL, cmax)[:, :SH_TOK].T.astype(np.float32)
        )
        e0, e1 = core_experts[core]
        for jslot, e in ((1, e0), (2, e1)):
            rows = tok_lists[e]
            if len(rows):
                ye = ytc[jslot].reshape(D_MODEL, cmax)[:, : len(rows)].T
                out[rows] += gate_lists[e][:, None] * ye.astype(np.float32)

    return out.reshape(b, s, d)


# revision 37
# speedup vs baseline: 1.4884x; 1.1983x over previous
"""DeepSeekMoE forward on 8 Trainium2 NeuronCores (Bass/Tile).

Strategy (expert-parallel, host dispatch/combine):
  - Router (sigmoid scores + top-4 + gating) computed on host with jax-CPU,
    bitwise-matching the reference's op sequence.
  - 24 uniform "FFN jobs": 16 routed experts (tokens gathered per expert,
    padded per-slot) + 2 shared experts x 4 token-shards of 2048.
  - Each core runs 3 jobs: 1 shared-expert shard + 2 routed experts,
    paired largest-with-smallest so slot capacities are
    [2048, C0=pad(max count), C1=pad(9th-largest count)] instead of
    2x global max.
  - Per job: H^T = relu(W1^T X^T + b1); Y^T = W2^T H^T + b2, computed with
    feature-major bf16 matmuls (full-rate on trn2 PE, fp32 PSUM accum;
    bf16 also enables fast-weight-load so LDWEIGHTS hides under MMs).
  - Host scatters routed outputs back with gating weights and adds the
    residual + shared outputs.
"""

import numpy as np

D_MODEL, D_FF, NS, NR, KR = 2048, 1408, 2, 16, 4
P = 128
DT = D_MODEL // P  # 16
FT = D_FF // P     # 11
NCORES = 8
JOBS = 3           # per core: [shared shard, routed expert big, routed small]
SH_TOK = 2048      # shared-expert shard size (per core)

_prog_cache = {}
LAST_RESULT = None  # BassKernelResults of the most recent device run


def _ensure_ntff_hook():
    """This image's `antenv` lacks the `axon_hooks` get/set registry that
    `run_bass_kernel_spmd(trace=True)` imports under axon; install an
    equivalent shim backed by the libaxon ctypes profiler so tracing works
    (and BASS_TRACE=1 doesn't crash the run)."""
    try:
        from antenv.axon_hooks import get_axon_ntff_profile_hook  # noqa: F401
        return
    except ImportError:
        pass
    import sys
    import types
    try:
        import antenv
        mod = types.ModuleType("antenv.axon_hooks")
        _hook = [None]
        mod.set_axon_ntff_profile_hook = lambda h: _hook.__setitem__(0, h)
        mod.get_axon_ntff_profile_hook = lambda: _hook[0]
        sys.modules["antenv.axon_hooks"] = mod
        antenv.axon_hooks = mod
        from trn_agent_boot.trn_boot import _ntff_profile_via_ctypes
        mod.set_axon_ntff_profile_hook(
            _ntff_profile_via_ctypes("/opt/axon/libaxon_pjrt.so")
        )
    except Exception:
        pass


def _plan_chunks(block):
    """Split a block (multiple of 128) into moving-dim chunks in
    {128, 256, 384, 512} (PSUM bank is 512 fp32; bigger chunks amortize
    per-MM issue overhead)."""
    n8 = block // P
    assert block % P == 0 and n8 >= 1
    out = []
    while n8 > 0:
        if n8 in (1, 2, 3, 4):
            out.append(n8 * P)
            n8 = 0
        elif n8 == 5:
            out += [2 * P, 3 * P]
            n8 = 0
        else:
            out.append(4 * P)
            n8 -= 4
    return out


def _plan_blocks(C, first_small=False):
    """Split capacity C into token blocks of at most 1152 (SBUF budget),
    each a multiple of 128. first_small peels a 256-token block off the
    front so the pipeline primes with minimal DMA."""
    blocks = []
    rem = C
    if first_small and C > 768:
        # big enough that mm1 (one ring's worth of w1) covers the weight
        # DMA of the block, small enough to prime the pipeline fast
        blocks.append(512)
        rem -= 512
    while rem > 0:
        if rem <= 1152:
            blocks.append(rem)
            rem = 0
        elif rem - 1024 >= 256:
            blocks.append(1024)
            rem -= 1024
        else:
            b = (rem // 2 // P) * P
            blocks += [b, rem - b]
            rem = 0
    assert sum(blocks) == C and all(b >= P and b % P == 0 for b in blocks)
    return blocks


WSCALE = 64.0  # fp8 weights are pre-scaled by this; undone in ACTIVATE


def _build_program(caps):
    import concourse.mybir as mybir
    import concourse.tile as tile
    from concourse import bacc

    F32 = mybir.dt.float32
    BF16 = mybir.dt.bfloat16
    FP8 = mybir.dt.float8e4
    DR = mybir.MatmulPerfMode.DoubleRow
    Relu = mybir.ActivationFunctionType.Relu
    Identity = mybir.ActivationFunctionType.Identity

    job_tokens = list(caps)
    cmax = max(job_tokens)
    job_blocks = [
        _plan_blocks(t, first_small=(j == 0)) for j, t in enumerate(job_tokens)
    ]

    # X is packed per block, [P, DT*blk] contiguous per partition, so every
    # X DMA moves 32KB-contiguous partition lines (1-2KB lines measured only
    # ~100-200 GB/s per ring; packed runs at full ring rate).
    # Shared job (0) runs bf16; routed jobs (1,2) run fp8e4 DoubleRow
    # (~1.4-1.7x PE throughput; verified 1.3e-2 rel err vs the 2e-2 gate).
    colsb = DT * sum(job_blocks[0])
    colsr = DT * (sum(job_blocks[1]) + sum(job_blocks[2]))
    nc = bacc.Bacc(None, target_bir_lowering=False)
    xtb = nc.dram_tensor("xtb", [P, colsb], BF16, kind="ExternalInput")
    xtr = nc.dram_tensor("xtr", [P, colsr], FP8, kind="ExternalInput")
    w1b = nc.dram_tensor("w1b", [FT, P, DT, P], BF16, kind="ExternalInput")
    w2b = nc.dram_tensor("w2b", [DT, P, FT, P], BF16, kind="ExternalInput")
    w1r = nc.dram_tensor("w1r", [2, FT, P, DT, P], FP8, kind="ExternalInput")
    w2r = nc.dram_tensor("w2r", [2, DT, P, FT, P], FP8, kind="ExternalInput")
    b1 = nc.dram_tensor("b1", [P, JOBS * FT], F32, kind="ExternalInput")
    b2 = nc.dram_tensor("b2", [P, JOBS * DT], F32, kind="ExternalInput")
    yt = nc.dram_tensor("yt", [JOBS, DT, P, cmax], BF16, kind="ExternalOutput")

    with tile.TileContext(nc) as tc:
        with (
            tc.tile_pool(name="const", bufs=1) as const,
            tc.tile_pool(name="h", bufs=1) as hpool,
            # weights stay SBUF-resident for a whole job (one load per
            # job, not per block): per-ft/dtile tags with bufs=1 give one
            # buffer per tile, so job j+1's tiles WAR-wait on job j's
            # last use and the transfers trickle in during job j's last
            # block.
            tc.tile_pool(name="w1p", bufs=1) as w1pool,
            tc.tile_pool(name="w2p", bufs=1) as w2pool,
            tc.tile_pool(name="y", bufs=4) as ypool,
            tc.tile_pool(name="ps", bufs=8, space="PSUM") as pspool,
        ):
            # biases ride the (otherwise idle) GPSIMD SW-DGE: tiny transfers
            # whose per-packet latency would delay block 0's X on a HW ring
            b1t = const.tile([P, JOBS * FT], F32)
            nc.gpsimd.dma_start(b1t[:], b1[:, :])
            b2t = const.tile([P, JOBS * DT], F32)
            nc.gpsimd.dma_start(b2t[:], b2[:, :])

            # HAM warm-up: the first real matmul can't start before its
            # 2MB X tile lands (~11us); idle-starting the PE then runs
            # everything at 1.2 GHz until ~3.4us of sustained busy flips
            # the clock gate to 8/8. Issue a burst of dummy matmuls with
            # NO dependencies (uninitialized SBUF into a scratch PSUM that
            # is never read) sized to end right as the X tile lands: the
            # gate is warm and the PE never idles. 22 MMs = ~3.4us cold +
            # ~3us warm after the ~7.7us engine preamble, ending just as
            # block 0's packed X lands (~14us).
            warm = const.tile([P, 512], BF16)
            nc.vector.memset(warm[:], 0.0)
            warm_ps = pspool.tile([P, 512], F32, tag="ps")
            for _ in range(22):
                nc.tensor.matmul(
                    warm_ps[:], warm[:, :P], warm[:], start=True, stop=True
                )

            def emit_block(j, off, blk, x_t, chunks, w1ts, w2ts, fp8):
                # Weight-stationary inner loops: each lhsT pair is loaded
                # once and all chunks' matmuls run under it — a DoubleRow
                # LDWEIGHTS (~370ns, 256 cols) only hides behind 2-3 MMs
                # (241ns each); chunk-major order exposed it on every MM.
                h_t = hpool.tile([P, FT, blk], FP8 if fp8 else BF16, tag="h")
                scale = (1.0 / WSCALE) if fp8 else 1.0
                coffs = []
                co = 0
                for ch in chunks:
                    coffs.append(co)
                    co += ch
                for ft in range(FT):
                    pss = [pspool.tile([P, 512], F32, tag="ps", name="ps")
                           for _ in chunks]
                    if fp8:
                        # DoubleRow: K=256 per MM via [P, 2, *] pairs
                        for pr in range(DT // 2):
                            for ps, ch, coff in zip(pss, chunks, coffs):
                                nc.tensor.matmul(
                                    ps[:, :ch],
                                    w1ts[ft][:, 2 * pr : 2 * pr + 2],
                                    x_t[:, 2 * pr : 2 * pr + 2,
                                        coff : coff + ch],
                                    start=(pr == 0),
                                    stop=(pr == DT // 2 - 1),
                                    perf_mode=DR,
                                )
                    else:
                        for ko in range(DT):
                            for ps, ch, coff in zip(pss, chunks, coffs):
                                nc.tensor.matmul(
                                    ps[:, :ch],
                                    w1ts[ft][:, ko],
                                    x_t[ko][:, coff : coff + ch],
                                    start=(ko == 0),
                                    stop=(ko == DT - 1),
                                )
                    for ps, ch, coff in zip(pss, chunks, coffs):
                        nc.scalar.activation(
                            h_t[:, ft, coff : coff + ch],
                            ps[:, :ch],
                            Relu,
                            bias=b1t[:, j * FT + ft : j * FT + ft + 1],
                            scale=scale,
                        )

                for dtile in range(DT):
                    y_t = ypool.tile([P, 1152], BF16, tag="y")
                    pss = [pspool.tile([P, 512], F32, tag="ps", name="ps")
                           for _ in chunks]
                    if fp8:
                        # FT=11: 5 DoubleRow pairs + 1 normal fp8 MM
                        for pr in range(FT // 2):
                            for ps, ch, coff in zip(pss, chunks, coffs):
                                nc.tensor.matmul(
                                    ps[:, :ch],
                                    w2ts[dtile][:, 2 * pr : 2 * pr + 2],
                                    h_t[:, 2 * pr : 2 * pr + 2,
                                        coff : coff + ch],
                                    start=(pr == 0),
                                    stop=False,
                                    perf_mode=DR,
                                )
                        for ps, ch, coff in zip(pss, chunks, coffs):
                            nc.tensor.matmul(
                                ps[:, :ch],
                                w2ts[dtile][:, FT - 1],
                                h_t[:, FT - 1, coff : coff + ch],
                                start=False,
                                stop=True,
                            )
                    else:
                        for ko in range(FT):
                            for ps, ch, coff in zip(pss, chunks, coffs):
                                nc.tensor.matmul(
                                    ps[:, :ch],
                                    w2ts[dtile][:, ko],
                                    h_t[:, ko, coff : coff + ch],
                                    start=(ko == 0),
                                    stop=(ko == FT - 1),
                                )
                    for ps, ch, coff in zip(pss, chunks, coffs):
                        nc.scalar.activation(
                            y_t[:, coff : coff + ch],
                            ps[:, :ch],
                            Identity,
                            bias=b2t[:, j * DT + dtile : j * DT + dtile + 1],
                            scale=scale,
                        )
                    # Y rides the ACT HW-DGE ring: keeps the SP ring free
                    # for weight transfers.
                    nc.scalar.dma_start(
                        yt[j, dtile, :, off : off + blk], y_t[:, :blk]
                    )

            # Steady state: X rides the SP ring, double-buffered (bufs=2);
            # w1 on SP, w2 on ACT (splits weight bandwidth), each loaded
            # ONCE per job and SBUF-resident across its blocks. Block 0's
            # X goes on the ACT ring (idle at start) so it lands in
            # parallel with job 0's w1 tiles on SP — one big descriptor,
            # since per-descriptor issue is ~0.6us and a fine-grained
            # priming split serializes on issue rate.
            with tc.tile_pool(name="x", bufs=2) as xpool:
                first = True
                xcol = 0
                for j in range(JOBS):
                    fp8 = j > 0
                    xsrc = xtr if fp8 else xtb
                    if fp8 and j == 1:
                        xcol = 0
                    # job 0, block 0: X is issued FIRST, split half/half
                    # across BOTH rings (ahead of the w1/w2 preloads), so
                    # the priming X lands at aggregate ring rate (~12.5us)
                    # instead of serializing on one ring.
                    xts_first = None
                    if first:
                        blk = job_blocks[0][0]
                        hd = DT // 2
                        x0 = xpool.tile([P, DT, blk], BF16, tag="x")
                        nc.scalar.dma_start(
                            x0[:, 0:hd], xtb[:, 0 : hd * blk]
                        )
                        nc.sync.dma_start(
                            x0[:, hd:DT], xtb[:, hd * blk : DT * blk]
                        )
                        xts_first = [x0[:, ko] for ko in range(DT)]
                        xcol = DT * blk
                    w1ts = []
                    w2ts = []
                    wdt = FP8 if fp8 else BF16
                    for ft in range(FT):
                        t = w1pool.tile([P, DT, P], wdt, tag=f"w1_{ft}")
                        src = w1r[j - 1, ft] if fp8 else w1b[ft]
                        nc.sync.dma_start(t[:], src)
                        w1ts.append(t)
                    for dtile in range(DT):
                        t = w2pool.tile([P, FT, P], wdt, tag=f"w2_{dtile}")
                        src = w2r[j - 1, dtile] if fp8 else w2b[dtile]
                        nc.scalar.dma_start(t[:], src)
                        w2ts.append(t)
                    off = 0
                    for blk in job_blocks[j]:
                        chunks = _plan_chunks(blk)
                        if xts_first is not None:
                            x_arg, xts_first = xts_first, None
                        else:
                            xt_t = xpool.tile(
                                [P, DT, blk], FP8 if fp8 else BF16, tag="x"
                            )
                            nc.sync.dma_start(
                                xt_t[:], xsrc[:, xcol : xcol + DT * blk]
                            )
                            xcol += DT * blk
                            x_arg = (
                                xt_t if fp8
                                else [xt_t[:, ko] for ko in range(DT)]
                            )
                        first = False
                        emit_block(
                            j, off, blk, x_arg, chunks, w1ts, w2ts, fp8,
                        )
                        off += blk

    nc.compile()
    return nc


def _routing(flat, centroids, bias):
    """Replicate the reference router bitwise: jax-CPU sigmoid scores,
    stable top-4 (argsort matches lax.top_k tie-breaking), normalized gates."""
    import jax
    import jax.numpy as jnp

    cpu = jax.devices("cpu")[0]
    with jax.default_device(cpu):
        scores = np.asarray(
            jax.nn.sigmoid(jnp.asarray(flat) @ jnp.asarray(centroids).T)
            + jnp.asarray(bias)
        )
    idx = np.argsort(-scores, axis=-1, kind="stable")[:, :KR]
    vals = np.take_along_axis(scores, idx, axis=-1)
    gating = vals / np.maximum(vals.sum(-1, keepdims=True, dtype=np.float32), 1e-8)
    return idx.astype(np.int32), gating.astype(np.float32)


def _feat_major(x_td):
    """[T, D] (rows=tokens) -> [P, D//P, T] feature-major device layout."""
    d = x_td.shape[1]
    return np.ascontiguousarray(x_td.T.reshape(d // P, P, -1).transpose(1, 0, 2))


def _w_tiles(w, kdim, mdim):
    """[K, M] -> [M//P, P(k_inner), K//P, P(m_inner)] lhsT tile layout."""
    kt, mt = kdim // P, mdim // P
    return np.ascontiguousarray(
        w.reshape(kt, P, mt, P).transpose(2, 1, 0, 3)
    )


def kernel(u, shared_w1, shared_b1, shared_w2, shared_b2,
           routed_w1, routed_b1, routed_w2, routed_b2, centroids, bias):
    import ml_dtypes
    from concourse.bass_utils import run_bass_kernel_spmd

    BF16 = np.dtype(ml_dtypes.bfloat16)

    _ensure_ntff_hook()
    u = np.asarray(u, dtype=np.float32)
    b, s, d = u.shape
    flat = u.reshape(-1, d)
    T = flat.shape[0]

    idx, gating = _routing(flat, np.asarray(centroids, np.float32),
                           np.asarray(bias, np.float32))

    # per-expert token lists (ascending token id) and their gate values
    tok_lists, gate_lists = [], []
    for e in range(NR):
        hit = idx == e                        # [T, KR]
        rows = np.nonzero(hit.any(axis=1))[0]
        g = gating[hit].reshape(-1)           # row-major -> ascending token id
        tok_lists.append(rows)
        gate_lists.append(g.astype(np.float32))

    # Pair largest with smallest so slot capacities are
    # C0 = pad(count of largest), C1 = pad(9th-largest count).
    counts = np.array([len(r) for r in tok_lists])
    order = np.argsort(-counts, kind="stable")
    pad = lambda c: max(256, -(-c // P) * P)
    C0 = pad(counts[order[0]])
    C1 = pad(counts[order[NCORES]])
    caps = (SH_TOK, C0, C1)
    cmax = max(caps)
    job_blocks = [
        _plan_blocks(t, first_small=(j == 0)) for j, t in enumerate(caps)
    ]
    total_cols = DT * sum(sum(bl) for bl in job_blocks)

    if caps not in _prog_cache:
        _prog_cache[caps] = _build_program(caps)
    nc = _prog_cache[caps]

    FP8 = np.dtype(ml_dtypes.float8_e4m3)
    flat_bf = flat.astype(BF16)
    flat_q8 = flat.astype(FP8)
    sw1 = np.asarray(shared_w1, np.float32).astype(BF16)
    sb1 = np.asarray(shared_b1, np.float32)
    sw2 = np.asarray(shared_w2, np.float32).astype(BF16)
    sb2 = np.asarray(shared_b2, np.float32)
    rw1 = (np.asarray(routed_w1, np.float32) * WSCALE).astype(FP8)
    rb1 = np.asarray(routed_b1, np.float32)
    rw2 = (np.asarray(routed_w2, np.float32) * WSCALE).astype(FP8)
    rb2 = np.asarray(routed_b2, np.float32)

    rw1_t = [_w_tiles(rw1[e], D_MODEL, D_FF) for e in range(NR)]
    rw2_t = [_w_tiles(rw2[e], D_FF, D_MODEL) for e in range(NR)]
    sw1_t = [_w_tiles(sw1[n], D_MODEL, D_FF) for n in range(NS)]
    sw2_t = [_w_tiles(sw2[n], D_FF, D_MODEL) for n in range(NS)]

    in_maps = []
    core_experts = []
    for core in range(NCORES):
        sh_e = core % NS
        sh_off = (core // NS) * SH_TOK
        e0 = int(order[core])                 # big expert -> slot 1 (C0)
        e1 = int(order[2 * NCORES - 1 - core])  # small expert -> slot 2 (C1)
        core_experts.append((e0, e1))

        fms = []
        fm0 = np.zeros((P, DT, SH_TOK), BF16)
        fm0[:] = _feat_major(flat_bf[sh_off : sh_off + SH_TOK])
        fms.append(fm0)
        for jslot, e in ((1, e0), (2, e1)):
            fmj = np.zeros((P, DT, caps[jslot]), FP8)
            rows = tok_lists[e]
            if len(rows):
                fmj[:, :, : len(rows)] = _feat_major(flat_q8[rows])
            fms.append(fmj)

        # pack X per block, [P, DT*blk] contiguous, in program block order
        xtb = np.empty((P, DT * caps[0]), BF16)
        xtr = np.empty((P, DT * (caps[1] + caps[2])), FP8)
        colb = colr = 0
        for j in range(JOBS):
            off = 0
            for blk in job_blocks[j]:
                sl = fms[j][:, :, off : off + blk].reshape(P, DT * blk)
                if j == 0:
                    xtb[:, colb : colb + DT * blk] = sl
                    colb += DT * blk
                else:
                    xtr[:, colr : colr + DT * blk] = sl
                    colr += DT * blk
                off += blk

        b1m = np.stack([sb1[sh_e], rb1[e0], rb1[e1]])   # [3, 1408]
        b2m = np.stack([sb2[sh_e], rb2[e0], rb2[e1]])   # [3, 2048]
        b1m = np.ascontiguousarray(b1m.reshape(JOBS * FT, P).T)  # [P, 33]
        b2m = np.ascontiguousarray(b2m.reshape(JOBS * DT, P).T)  # [P, 48]

        in_maps.append({
            "xtb": xtb, "xtr": xtr,
            "w1b": sw1_t[sh_e], "w2b": sw2_t[sh_e],
            "w1r": np.stack([rw1_t[e0], rw1_t[e1]]),
            "w2r": np.stack([rw2_t[e0], rw2_t[e1]]),
            "b1": b1m, "b2": b2m,
        })

    res = run_bass_kernel_spmd(nc, in_maps, core_ids=list(range(NCORES)))
    global LAST_RESULT
    LAST_RESULT = res

    out = flat.copy()
    for core in range(NCORES):
        ytc = res.results[core]["yt"]          # [JOBS, DT, P, cmax] bf16
        sh_off = (core // NS) * SH_TOK
        out[sh_off : sh_off + SH_TOK] += (
            ytc[0].reshape(D_MODEL, cmax)[:, :SH_TOK].T.astype(np.float32)
        )
        e0, e1 = core_experts[core]
        for jslot, e in ((1, e0), (2, e1)):
            rows = tok_lists[e]
            if len(rows):
                ye = ytc[jslot].reshape(D_MODEL, cmax)[:, : len(rows)].T
                out[rows] += gate_lists[e][:, None] * ye.astype(np.float32)

    return out.reshape(b, s, d)
